# revision 1
# baseline (speedup 1.0000x reference)
"""Cross-attention kernel for Trainium2, 8 NeuronCores.

Sharding (data + head parallel, per the problem's sharding hint):
  core c in 0..7 -> batch b = c // 4, head-pair hp = c % 4.
  Each core computes attention for its batch with 2 of the 8 heads
  (a 128-wide slice of the 512 hidden features), then the partial
  out-projection  attn_out_slice @ Wo[slice, :].  The host sums the 4
  partials per batch (the "all-reduce") ; bo is added on the hp==0 core.

Device-side dataflow per core (all matmuls fp32r, feature-major):
  qT[128, N] = Wq_sl.T @ x.T          (contraction over D=1024 in 8 chunks)
  kT[128, M] = Wk_sl.T @ ctx.T
  vT[128, M] = Wv_sl.T @ ctx.T
  V_aug[m,65] = PE-transpose of vT per head + ones column
  per n-chunk s (512 cols), per m-chunk mc (128 rows):
     St[m 128, n 1024] = [kT_h0_mc.T @ qT_h0_s | kT_h1_mc.T @ qT_h1_s]
         (two concurrent matmuls on PE row-groups 0-63 / 64-127)
     Pt = exp(St * 1/8)               (ScalarE, one op per m-chunk)
     Oaug_h[65, 512] += V_aug_h_mc.T @ Pt_h                (PSUM accum)
  row 64 of Oaug = softmax denominators; OT[h*64:, s] = Oaug[0:64]/denom
  out_p[n 128, 1024] = OT_ntile.T @ Wo_sl + bo             (per n-tile)
"""

import numpy as np

import concourse.bass as bass
import concourse.tile as tile
from concourse import bacc, mybir
from concourse.masks import make_identity

F32 = mybir.dt.float32
F32R = mybir.dt.float32r
BF16 = mybir.dt.bfloat16

USE_BF16 = True          # activation/weight dtype for all matmuls
PROBE_ST_F32R = False    # debug: keep the S^T matmul inputs fp32r
PROBE_PV_F32R = False    # debug: keep PV + outproj inputs fp32r
VPAD = 72                # PV weight row padded to 16B-aligned stride (bf16)
AT = BF16 if USE_BF16 else F32R

D = 1024      # model dim (contraction for projections)
SEQ = 2048    # n == m
F = 128       # features per core (2 heads x 64)
DH = 64       # head dim
NS = SEQ // 512   # 4 n-chunks of 512
NK = D // 128     # 8 contraction chunks
NM = SEQ // 128   # 16 m-chunks of 128
SCALE = DH ** -0.5


def build_nc():
    nc = bacc.Bacc("TRN2", target_bir_lowering=False, debug=False)

    xT_d = nc.dram_tensor("xT", [D, SEQ], AT, kind="ExternalInput")
    cT_d = nc.dram_tensor("cT", [D, SEQ], AT, kind="ExternalInput")
    # wq/wk/wv arrive pre-swizzled by the host: [128, NK*128] where
    # column block k holds W[k*128:(k+1)*128, :].T-chunk laid contiguously.
    wq_d = nc.dram_tensor("wq", [128, NK * 128], AT, kind="ExternalInput")
    wk_d = nc.dram_tensor("wk", [128, NK * 128], AT, kind="ExternalInput")
    wv_d = nc.dram_tensor("wv", [128, NK * 128], AT, kind="ExternalInput")
    WO_DT = F32R if PROBE_PV_F32R else AT
    wo_d = nc.dram_tensor("wo", [F, D], WO_DT, kind="ExternalInput")
    bo_d = nc.dram_tensor("bo", [1, D], F32, kind="ExternalInput")
    out_d = nc.dram_tensor("out_p", [SEQ, D], F32, kind="ExternalOutput")

    with tile.TileContext(nc) as tc:
        _emit(tc, nc, xT_d, cT_d, wq_d, wk_d, wv_d, wo_d, bo_d, out_d)
    nc.compile()
    return nc


def _emit(tc, nc, xT_d, cT_d, wq_d, wk_d, wv_d, wo_d, bo_d, out_d):
    from contextlib import ExitStack

    ctx = ExitStack()
    wpool = ctx.enter_context(tc.tile_pool(name="wpool", bufs=1))
    big = ctx.enter_context(tc.tile_pool(name="big", bufs=1))
    stream = ctx.enter_context(tc.tile_pool(name="stream", bufs=10))
    ptp = ctx.enter_context(tc.tile_pool(name="ptp", bufs=4))
    ostage = ctx.enter_context(tc.tile_pool(name="ostage", bufs=4))
    dscr = ctx.enter_context(tc.tile_pool(name="dscr", bufs=2, space="DRAM"))
    ps_small = ctx.enter_context(tc.tile_pool(name="ps_small", bufs=2, space="PSUM"))
    ps_st = ctx.enter_context(tc.tile_pool(name="ps_st", bufs=2, space="PSUM"))
    ps_oaug = ctx.enter_context(tc.tile_pool(name="ps_oaug", bufs=2, space="PSUM"))

    # ---- constants / weights (contiguous DMAs; host pre-swizzled) ----
    # Per-chunk DMA pieces so they spread across DMA queues (one dma_start
    # lands on a single queue at ~32 GB/s; splitting cuts arrival latency).
    def load_w(w_s, w_d):
        for k in range(NK):
            nc.sync.dma_start(
                out=w_s[:, k, :], in_=w_d.ap()[:, k * 128 : (k + 1) * 128]
            )

    wq_s = wpool.tile([128, NK, 128], AT, name="wq_s")
    wk_s = wpool.tile([128, NK, 128], AT, name="wk_s")
    wv_s = wpool.tile([128, NK, 128], AT, name="wv_s")
    load_w(wq_s, wq_d)
    wo_s = wpool.tile([128, D], F32R if PROBE_PV_F32R else AT, name="wo_s")
    bo_rep = wpool.tile([128, D], F32, name="bo_rep")
    ident = wpool.tile([128, DH], F32, name="ident")
    make_identity(nc, ident[0:DH, :])
    make_identity(nc, ident[DH:128, :])
    zbias = wpool.tile([128, 1], F32, name="zbias")
    nc.vector.memset(zbias, 0.0)

    ST_DT = F32R if PROBE_ST_F32R else AT
    qT = big.tile([128, SEQ], ST_DT, name="qT", tag="qT")
    kT = big.tile([128, SEQ], ST_DT, name="kT", tag="kT")
    vT = big.tile([128, SEQ], F32, name="vT", tag="vT")
    PV_DT = F32R if PROBE_PV_F32R else AT
    OT = big.tile([128, SEQ], PV_DT, name="OT", tag="OT")
    # V per head+m-chunk, with a ones column (65th) that accumulates the
    # softmax denominators during the PV matmul.
    Vall = big.tile([128, 2, NM, VPAD], PV_DT, name="Vall", tag="Vall")
    ones_sb = wpool.tile([128, 2 * NM], F32, name="ones_sb")
    nc.vector.memset(ones_sb, 1.0)
    nc.vector.tensor_copy(
        out=Vall[:, :, :, DH : DH + 1],
        in_=ones_sb.rearrange("p (h m o) -> p h m o", h=2, o=1),
    )
    if VPAD > DH + 1:
        zpad_sb = wpool.tile([128, 2 * NM * (VPAD - DH - 1)], F32, name="zpad_sb")
        nc.vector.memset(zpad_sb, 0.0)
        nc.vector.tensor_copy(
            out=Vall[:, :, :, DH + 1 :],
            in_=zpad_sb.rearrange("p (h m o) -> p h m o", h=2, m=NM),
        )

    def load_chunk(src_d, k, s):
        """[128, 512] activation chunk as 2 DMA pieces on separate queues."""
        chunk = stream.tile([128, 512], AT, name="chunk", tag="stream")
        for p in range(2):
            nc.sync.dma_start(
                out=chunk[:, p * 256 : (p + 1) * 256],
                in_=src_d.ap()[
                    k * 128 : (k + 1) * 128,
                    s * 512 + p * 256 : s * 512 + (p + 1) * 256,
                ],
            )
        return chunk

    def proj(dst, w_s, src_d, s):
        """dst[:, s*512:(s+1)*512] = w_s.T @ src_d (contraction over D)."""
        acc = ps_small.tile([128, 512], F32, name="proj_acc", tag="small")
        for k in range(NK):
            chunk = load_chunk(src_d, k, s)
            nc.tensor.matmul(
                acc, w_s[:, k, :], chunk,
                start=(k == 0), stop=(k == NK - 1),
            )
        nc.vector.tensor_copy(out=dst[:, s * 512 : (s + 1) * 512], in_=acc)

    def kv_proj(g):
        """k and v projections for seq chunk g (they share the ctx stream)."""
        kacc = ps_small.tile([128, 512], F32, name="kacc", tag="small")
        vacc = ps_small.tile([128, 512], F32, name="vacc", tag="small")
        for k in range(NK):
            chunk = load_chunk(cT_d, k, g)
            nc.tensor.matmul(
                kacc, wk_s[:, k, :], chunk,
                start=(k == 0), stop=(k == NK - 1),
            )
            nc.tensor.matmul(
                vacc, wv_s[:, k, :], chunk,
                start=(k == 0), stop=(k == NK - 1),
            )
        nc.vector.tensor_copy(out=kT[:, g * 512 : (g + 1) * 512], in_=kacc)
        nc.vector.tensor_copy(out=vT[:, g * 512 : (g + 1) * 512], in_=vacc)

    def v_transpose(g):
        """Vall[:, h, mc, 0:64] = vT[h*64:(h+1)*64, mc*128:(mc+1)*128].T"""
        for mc in range(4 * g, 4 * g + 4):
            for h in range(2):
                tp = ps_small.tile([128, DH], F32, name="tp", tag="small")
                nc.tensor.transpose(
                    tp,
                    vT[h * DH : (h + 1) * DH, mc * 128 : (mc + 1) * 128],
                    ident[h * DH : (h + 1) * DH, :],
                )
                nc.vector.tensor_copy(out=Vall[:, h, mc, 0:DH], in_=tp)

    def attn_group(s, oaug, mcs):
        """Attention for n-chunk s over the given m-chunks."""
        n0, n1 = s * 512, (s + 1) * 512
        for mc in mcs:
            m0, m1 = mc * 128, (mc + 1) * 128
            st = ps_st.tile([128, 1024], F32, name="st", tag="st")
            nc.tensor.matmul(
                st[:, 0:512], kT[0:DH, m0:m1], qT[0:DH, n0:n1],
                start=True, stop=True, tile_position=(0, 0),
            )
            nc.tensor.matmul(
                st[:, 512:1024], kT[DH:128, m0:m1], qT[DH:128, n0:n1],
                start=True, stop=True, tile_position=(64, 0),
            )
            pt = ptp.tile([128, 1024], PV_DT, name="pt", tag="pt")
            nc.scalar.activation(
                out=pt, in_=st,
                func=mybir.ActivationFunctionType.Exp,
                bias=zbias, scale=SCALE,
            )
            nc.tensor.matmul(
                oaug[0], Vall[:, 0, mc, 0 : DH + 1], pt[:, 0:512],
                start=(mc == 0), stop=(mc == NM - 1),
            )
            nc.tensor.matmul(
                oaug[1], Vall[:, 1, mc, 0 : DH + 1], pt[:, 512:1024],
                start=(mc == 0), stop=(mc == NM - 1),
            )

    def fin(s, oaug):
        """Normalize by softmax denominators (row 64 of oaug) into OT.

        The PSUM accumulators are evacuated to SBUF immediately so the
        banks free up for the next n-chunk's accumulation.  The [1, 512]
        denominator row is repartitioned to [128, 4] via a DRAM bounce so
        the reciprocal runs on all DVE lanes (a single-partition
        reciprocal measures ~3.3 us; this way it is ~30 ns + small DMAs).
        """
        n0, n1 = s * 512, (s + 1) * 512
        for h in range(2):
            oaug_sb = ostage.tile([DH + 1, 512], F32, name="oaug_sb",
                                  tag="oaug_sb", bufs=2)
            nc.vector.tensor_copy(out=oaug_sb, in_=oaug[h])
            den_p = ostage.tile([128, 4], F32, name="den_p", tag="den_p", bufs=2)
            nc.sync.dma_start(out=den_p, in_=oaug_sb[DH : DH + 1, :])
            rec_p = ostage.tile([128, 4], F32, name="rec_p", tag="rec_p", bufs=2)
            nc.vector.reciprocal(out=rec_p, in_=den_p)
            scr2 = dscr.tile([128, 4], F32, name="scr2", tag="scr2")
            nc.sync.dma_start(out=scr2, in_=rec_p)
            recip_rep = ostage.tile([DH, 512], F32, name="recip_rep",
                                    tag="recip_rep", bufs=2)
            nc.sync.dma_start(
                out=recip_rep,
                in_=scr2.rearrange("p f -> (p f)").partition_broadcast(DH),
            )
            nc.vector.tensor_mul(
                out=OT[h * DH : (h + 1) * DH, n0:n1],
                in0=oaug_sb[0:DH, :],
                in1=recip_rep,
            )

    def outproj(s):
        for t in range(4):
            nt = s * 4 + t
            for half in range(2):
                c0, c1 = half * 512, (half + 1) * 512
                ops = ps_small.tile([128, 512], F32, name="ops", tag="small")
                nc.tensor.matmul(
                    ops, OT[:, nt * 128 : (nt + 1) * 128], wo_s[:, c0:c1],
                    start=True, stop=True,
                )
                osb = ostage.tile([128, 512], F32, name="osb", tag="osb")
                nc.vector.tensor_add(out=osb, in0=ops, in1=bo_rep[:, c0:c1])
                nc.sync.dma_start(
                    out=out_d.ap()[nt * 128 : (nt + 1) * 128, c0:c1], in_=osb
                )

    # ---- schedule ----
    # q(s=0) first so attention over n-chunk 0 can start as soon as the
    # first k/v seq-chunk lands; kv groups stream in and attention(s=0)
    # chases them m-group by m-group.
    def mk_oaug(s):
        return [
            ps_oaug.tile([DH + 1, 512], F32, name=f"oaug{s}_{h}", tag="oaug")
            for h in range(2)
        ]

    proj(qT, wq_s, xT_d, 0)
    load_w(wk_s, wk_d)
    load_w(wv_s, wv_d)
    oaug_cur = mk_oaug(0)
    for g in range(NS):
        kv_proj(g)
        v_transpose(g)
        attn_group(0, oaug_cur, list(range(4 * g, 4 * g + 4)))
        if g == 0:
            # deferred so the big streaming DMAs win the early queue slots
            load_w(wo_s.rearrange("p (c f) -> p c f", c=NK), wo_d)
            nc.gpsimd.dma_start(
                out=bo_rep, in_=bo_d.ap()[0, :].partition_broadcast(128)
            )
    for s in range(1, NS):
        # next n-chunk's q projection first: it keeps PE busy while the
        # previous chunk's fin/outproj chain (small DMAs) resolves.
        proj(qT, wq_s, xT_d, s)
        oaug_next = mk_oaug(s)
        fin(s - 1, oaug_cur)
        attn_group(s, oaug_next, list(range(NM)))
        outproj(s - 1)
        oaug_cur = oaug_next
    fin(NS - 1, oaug_cur)
    outproj(NS - 1)

    ctx.close()


_NC = None


def _get_nc():
    global _NC
    if _NC is None:
        _NC = build_nc()
    return _NC


def _np_at():
    if USE_BF16:
        import ml_dtypes

        return ml_dtypes.bfloat16
    return np.float32


def _swizzle(w):
    """[1024, 128] -> [128, 8*128]: chunk k of the contraction dim lands in
    column block k, so the device DMA is fully contiguous."""
    return np.ascontiguousarray(
        np.asarray(w, np.float32).reshape(NK, 128, F).transpose(1, 0, 2)
        .reshape(128, NK * F).astype(_np_at())
    )


def shard_inputs(x, context, Wq, Wk, Wv, Wo, bo):
    x = np.asarray(x, np.float32)
    context = np.asarray(context, np.float32)
    Wq = np.asarray(Wq, np.float32)
    Wk = np.asarray(Wk, np.float32)
    Wv = np.asarray(Wv, np.float32)
    Wo = np.asarray(Wo, np.float32)
    bo = np.asarray(bo, np.float32)

    at = _np_at()
    xT = [np.ascontiguousarray(x[b].T).astype(at) for b in range(x.shape[0])]
    cT = [np.ascontiguousarray(context[b].T).astype(at) for b in range(context.shape[0])]
    zero_bo = np.zeros((1, D), np.float32)
    in_maps = []
    for c in range(8):
        b, hp = divmod(c, 4)
        f0 = hp * F
        in_maps.append(
            {
                "xT": xT[b],
                "cT": cT[b],
                "wq": _swizzle(Wq[:, f0 : f0 + F]),
                "wk": _swizzle(Wk[:, f0 : f0 + F]),
                "wv": _swizzle(Wv[:, f0 : f0 + F]),
                "wo": np.ascontiguousarray(Wo[f0 : f0 + F, :]).astype(
                    np.float32 if PROBE_PV_F32R else _np_at()
                ),
                "bo": bo.reshape(1, D) if hp == 0 else zero_bo,
            }
        )
    return in_maps


def kernel(x, context, Wq, Wk, Wv, Wo, bo):
    from concourse.bass_utils import run_bass_kernel_spmd

    in_maps = shard_inputs(x, context, Wq, Wk, Wv, Wo, bo)
    nc = _get_nc()
    res = run_bass_kernel_spmd(nc, in_maps, list(range(8)))
    out = np.zeros((2, SEQ, D), np.float32)
    for c in range(8):
        out[c // 4] += res.results[c]["out_p"]
    return out



# revision 2
# speedup vs baseline: 1.1415x; 1.1415x over previous
"""Cross-attention kernel for Trainium2, 8 NeuronCores.

Sharding (data + head parallel, per the problem's sharding hint):
  core c in 0..7 -> batch b = c // 4, head-pair hp = c % 4.
  Each core computes attention for its batch with 2 of the 8 heads
  (a 128-wide slice of the 512 hidden features), then the partial
  out-projection  attn_out_slice @ Wo[slice, :].  The host sums the 4
  partials per batch (the "all-reduce"); bo is added on the hp==0 core.

Differences from the first working version (191 us):
  - Inputs land in SBUF via 17 LARGE DMAs (wqkv + 8 x-chunks + 8
    c-chunks, all on the sync HWDGE ring, in that order) instead of
    ~180 small ones: the old kernel was gated by per-dma_start issue
    cost (~0.6 us each) on the Sync engine for its first 40 us.
  - x loads before context so the q(s=0) projection (which contracts
    over ALL of x) finishes as early as possible; attention on n-chunk
    0 then chases the k/v projections group by group.
  - Dummy matmuls at t=0 warm the PE HAM clock gate (the old kernel ran
    its first 55 us at K=4/8 = 1.2 GHz).
  - fin() avoids DRAM round trips: SBUF->SBUF repartition DMAs + a PE
    broadcast matmul (ones[1,64].T @ recip_row) replace the two
    DRAM bounces.
  - Output stores go out on the (otherwise idle) gpsimd SWDGE queue.
  - outproj(s-1) is issued mid-attn(s) so only outproj(3) is in the tail.
"""

import numpy as np

import concourse.bass as bass
import concourse.tile as tile
from concourse import bacc, mybir
from concourse.masks import make_identity

F32 = mybir.dt.float32
BF16 = mybir.dt.bfloat16
F8 = mybir.dt.float8e4

USE_FP8_INPUTS = False   # x/context/Wqkv in fp8e4m3 (halves input DMA)
AT = F8 if USE_FP8_INPUTS else BF16
VPAD = 72                # PV weight row padded to 16B-aligned stride (bf16)

D = 1024      # model dim (contraction for projections)
SEQ = 2048    # n == m
F = 128       # features per core (2 heads x 64)
DH = 64       # head dim
NS = SEQ // 512   # 4 n-chunks of 512
NK = D // 128     # 8 contraction chunks
NM = SEQ // 128   # 16 m-chunks of 128
SCALE = DH ** -0.5
N_WARM = 20       # HAM warm-up matmuls


def build_nc():
    nc = bacc.Bacc("TRN2", target_bir_lowering=False, debug=False)

    xT_d = nc.dram_tensor("xT", [D, SEQ], AT, kind="ExternalInput")
    cT_d = nc.dram_tensor("cT", [D, SEQ], AT, kind="ExternalInput")
    # host-packed: [128, 3*NK*128]; block (w, k) holds W_w[k*128:(k+1)*128, :]
    # with the chunk's rows on the partition axis.
    wqkv_d = nc.dram_tensor("wqkv", [128, 3 * NK * 128], AT, kind="ExternalInput")
    wo_d = nc.dram_tensor("wo", [F, D], BF16, kind="ExternalInput")
    bo_d = nc.dram_tensor("bo", [1, D], F32, kind="ExternalInput")
    out_d = nc.dram_tensor("out_p", [SEQ, D], F32, kind="ExternalOutput")

    with tile.TileContext(nc) as tc:
        _emit(tc, nc, xT_d, cT_d, wqkv_d, wo_d, bo_d, out_d)
    nc.compile()
    return nc


def _emit(tc, nc, xT_d, cT_d, wqkv_d, wo_d, bo_d, out_d):
    from contextlib import ExitStack

    ctx = ExitStack()
    wpool = ctx.enter_context(tc.tile_pool(name="wpool", bufs=1))
    big = ctx.enter_context(tc.tile_pool(name="big", bufs=1))
    ptp = ctx.enter_context(tc.tile_pool(name="ptp", bufs=4))
    ostage = ctx.enter_context(tc.tile_pool(name="ostage", bufs=2))
    # PSUM budget (8 banks x 2KB):
    #   st ring  : 2 x [128,1024] f32 = 4 banks
    #   oaug ring: 2 x [65,512]  f32 = 2 banks
    #   acc ring : 2 x [128,512] f32 = 2 banks (kacc/vacc/qacc/tp/bc/ops)
    ps_st = ctx.enter_context(tc.tile_pool(name="ps_st", bufs=2, space="PSUM"))
    ps_acc = ctx.enter_context(tc.tile_pool(name="ps_acc", bufs=2, space="PSUM"))
    ps_oaug = ctx.enter_context(tc.tile_pool(name="ps_oaug", bufs=2, space="PSUM"))

    # ---- constants ----
    ident = wpool.tile([128, 128], BF16, name="ident")
    make_identity(nc, ident)
    zbias = wpool.tile([128, 1], F32, name="zbias")
    nc.vector.memset(zbias, 0.0)
    ones64 = wpool.tile([1, DH], F32, name="ones64")
    nc.vector.memset(ones64, 1.0)
    warm = wpool.tile([128, 128], BF16, name="warm")
    nc.vector.memset(warm, 0.0)

    # ---- input DMAs: one sync-ring queue, program order = arrival order ----
    wqkv_s = wpool.tile([128, 3, NK, 128], AT, name="wqkv_s")
    nc.sync.dma_start(out=wqkv_s.rearrange("p a b c -> p (a b c)"), in_=wqkv_d.ap())
    xS = wpool.tile([128, NK, SEQ], AT, name="xS")
    for k in range(NK):
        nc.sync.dma_start(out=xS[:, k, :], in_=xT_d.ap()[k * 128 : (k + 1) * 128, :])
    cS = wpool.tile([128, NK, SEQ], AT, name="cS")
    for k in range(NK):
        nc.sync.dma_start(out=cS[:, k, :], in_=cT_d.ap()[k * 128 : (k + 1) * 128, :])
    wo_s = wpool.tile([128, D], BF16, name="wo_s")
    nc.sync.dma_start(out=wo_s, in_=wo_d.ap())
    bo_rep = wpool.tile([128, D], F32, name="bo_rep")
    nc.gpsimd.dma_start(out=bo_rep, in_=bo_d.ap()[0, :].partition_broadcast(128))

    # ---- persistent activations ----
    qT = big.tile([128, SEQ], BF16, name="qT", tag="qT")
    kT = big.tile([128, SEQ], BF16, name="kT", tag="kT")
    vT = big.tile([128, SEQ], BF16, name="vT", tag="vT")
    OT = big.tile([128, SEQ], BF16, name="OT", tag="OT")
    # V per head+m-chunk, with a ones column (65th) that accumulates the
    # softmax denominators during the PV matmul.
    Vall = big.tile([128, 2, NM, VPAD], BF16, name="Vall", tag="Vall")
    ones_sb = wpool.tile([128, 2 * NM], F32, name="ones_sb")
    nc.vector.memset(ones_sb, 1.0)
    nc.vector.tensor_copy(
        out=Vall[:, :, :, DH : DH + 1],
        in_=ones_sb.rearrange("p (h m o) -> p h m o", h=2, o=1),
    )
    zpad_sb = wpool.tile([128, 2 * NM * (VPAD - DH - 1)], F32, name="zpad_sb")
    nc.vector.memset(zpad_sb, 0.0)
    nc.vector.tensor_copy(
        out=Vall[:, :, :, DH + 1 :],
        in_=zpad_sb.rearrange("p (h m o) -> p h m o", h=2, m=NM),
    )

    # ---- HAM warm-up: dummy matmuls with no DMA deps ----
    wps = ps_st.tile([128, 1024], F32, name="wps", tag="st")
    for _ in range(N_WARM):
        nc.tensor.matmul(wps[:, 0:128], warm, warm, start=True, stop=True)

    def qproj(s):
        """qT[:, s*512:(s+1)*512] = Wq_sl.T @ x.T (chases the xS DMAs)."""
        qacc = ps_acc.tile([128, 512], F32, name="qacc", tag="acc")
        for k in range(NK):
            nc.tensor.matmul(
                qacc, wqkv_s[:, 0, k, :], xS[:, k, s * 512 : (s + 1) * 512],
                start=(k == 0), stop=(k == NK - 1),
            )
        nc.vector.tensor_copy(out=qT[:, s * 512 : (s + 1) * 512], in_=qacc)

    def kvproj(g):
        kacc = ps_acc.tile([128, 512], F32, name="kacc", tag="acc")
        vacc = ps_acc.tile([128, 512], F32, name="vacc", tag="acc")
        for k in range(NK):
            nc.tensor.matmul(
                kacc, wqkv_s[:, 1, k, :], cS[:, k, g * 512 : (g + 1) * 512],
                start=(k == 0), stop=(k == NK - 1),
            )
            nc.tensor.matmul(
                vacc, wqkv_s[:, 2, k, :], cS[:, k, g * 512 : (g + 1) * 512],
                start=(k == 0), stop=(k == NK - 1),
            )
        nc.vector.tensor_copy(out=kT[:, g * 512 : (g + 1) * 512], in_=kacc)
        nc.vector.tensor_copy(out=vT[:, g * 512 : (g + 1) * 512], in_=vacc)

    def vtrans(g):
        """Vall[:, h, mc, 0:64] = vT[h*64:(h+1)*64, mc*128:(mc+1)*128].T
        Both heads in one [128,128] PE transpose."""
        for mc in range(4 * g, 4 * g + 4):
            tp = ps_acc.tile([128, 128], BF16, name="tp", tag="acc")
            nc.tensor.transpose(tp, vT[:, mc * 128 : (mc + 1) * 128], ident)
            nc.vector.tensor_copy(
                out=Vall[:, :, mc, 0:DH],
                in_=tp.rearrange("p (h d) -> p h d", h=2),
            )

    def attn(s, oaug, mcs):
        """Attention for n-chunk s over the given m-chunks."""
        n0, n1 = s * 512, (s + 1) * 512
        for mc in mcs:
            m0, m1 = mc * 128, (mc + 1) * 128
            st = ps_st.tile([128, 1024], F32, name="st", tag="st")
            nc.tensor.matmul(
                st[:, 0:512], kT[0:DH, m0:m1], qT[0:DH, n0:n1],
                start=True, stop=True, tile_position=(0, 0),
            )
            nc.tensor.matmul(
                st[:, 512:1024], kT[DH:128, m0:m1], qT[DH:128, n0:n1],
                start=True, stop=True, tile_position=(64, 0),
            )
            pt = ptp.tile([128, 1024], BF16, name="pt", tag="pt")
            nc.scalar.activation(
                out=pt, in_=st,
                func=mybir.ActivationFunctionType.Exp,
                bias=zbias, scale=SCALE,
            )
            nc.tensor.matmul(
                oaug[0], Vall[:, 0, mc, 0 : DH + 1], pt[:, 0:512],
                start=(mc == 0), stop=(mc == NM - 1),
            )
            nc.tensor.matmul(
                oaug[1], Vall[:, 1, mc, 0 : DH + 1], pt[:, 512:1024],
                start=(mc == 0), stop=(mc == NM - 1),
            )

    def fin(s, oaug):
        """Normalize by softmax denominators (row 64 of oaug) into OT.

        The [1,512] denominator row is repartitioned to [128,4] with an
        SBUF->SBUF DMA so the reciprocal runs on all DVE lanes, DMA'd
        back to a [1,512] row, and broadcast to 64 partitions with a PE
        matmul (ones[1,64].T @ rec_row) -- no DRAM round trips.
        """
        n0, n1 = s * 512, (s + 1) * 512
        for h in range(2):
            oaug_sb = ostage.tile([DH + 1, 512], F32, name="oaug_sb",
                                  tag="oaug_sb", bufs=2)
            nc.vector.tensor_copy(out=oaug_sb, in_=oaug[h])
            den_p = ostage.tile([128, 4], F32, name="den_p", tag="den_p", bufs=2)
            nc.sync.dma_start(out=den_p, in_=oaug_sb[DH : DH + 1, :])
            rec_p = ostage.tile([128, 4], F32, name="rec_p", tag="rec_p", bufs=2)
            nc.vector.reciprocal(out=rec_p, in_=den_p)
            rec_row = ostage.tile([1, 512], F32, name="rec_row", tag="rec_row",
                                  bufs=2)
            nc.sync.dma_start(out=rec_row, in_=rec_p)
            bc = ps_acc.tile([DH, 512], F32, name="bc", tag="acc")
            nc.tensor.matmul(bc, ones64, rec_row, start=True, stop=True)
            nc.vector.tensor_mul(
                out=OT[h * DH : (h + 1) * DH, n0:n1],
                in0=oaug_sb[0:DH, :],
                in1=bc,
            )

    def outproj(s):
        for t in range(4):
            nt = s * 4 + t
            for half in range(2):
                c0, c1 = half * 512, (half + 1) * 512
                ops = ps_acc.tile([128, 512], F32, name="ops", tag="acc")
                nc.tensor.matmul(
                    ops, OT[:, nt * 128 : (nt + 1) * 128], wo_s[:, c0:c1],
                    start=True, stop=True,
                )
                osb = ostage.tile([128, 512], F32, name="osb", tag="osb", bufs=4)
                nc.vector.tensor_add(out=osb, in0=ops, in1=bo_rep[:, c0:c1])
                nc.gpsimd.dma_start(
                    out=out_d.ap()[nt * 128 : (nt + 1) * 128, c0:c1], in_=osb
                )

    # ---- schedule ----
    def mk_oaug(s):
        return [
            ps_oaug.tile([DH + 1, 512], F32, name=f"oaug{s}_{h}", tag="oaug")
            for h in range(2)
        ]

    qproj(0)
    oaug_cur = mk_oaug(0)
    for g in range(NS):
        kvproj(g)
        vtrans(g)
        attn(0, oaug_cur, list(range(4 * g, 4 * g + 4)))
    for s in range(1, NS):
        qproj(s)
        oaug_next = mk_oaug(s)
        fin(s - 1, oaug_cur)
        attn(s, oaug_next, list(range(0, 8)))
        outproj(s - 1)
        attn(s, oaug_next, list(range(8, NM)))
        oaug_cur = oaug_next
    fin(NS - 1, oaug_cur)
    outproj(NS - 1)

    ctx.close()


_NC = None


def _get_nc():
    global _NC
    if _NC is None:
        _NC = build_nc()
    return _NC


def _np_at():
    import ml_dtypes

    return ml_dtypes.float8_e4m3 if USE_FP8_INPUTS else ml_dtypes.bfloat16


def _swizzle(w):
    """[1024, 128] -> [128, 8*128]: chunk k of the contraction dim lands in
    column block k with the chunk's rows on the partition axis."""
    return (
        np.asarray(w, np.float32).reshape(NK, 128, F).transpose(1, 0, 2)
        .reshape(128, NK * F)
    )


def shard_inputs(x, context, Wq, Wk, Wv, Wo, bo):
    import ml_dtypes

    x = np.asarray(x, np.float32)
    context = np.asarray(context, np.float32)
    Wq = np.asarray(Wq, np.float32)
    Wk = np.asarray(Wk, np.float32)
    Wv = np.asarray(Wv, np.float32)
    Wo = np.asarray(Wo, np.float32)
    bo = np.asarray(bo, np.float32)

    at = _np_at()
    xT = [np.ascontiguousarray(x[b].T).astype(at) for b in range(x.shape[0])]
    cT = [np.ascontiguousarray(context[b].T).astype(at) for b in range(context.shape[0])]
    zero_bo = np.zeros((1, D), np.float32)
    in_maps = []
    for c in range(8):
        b, hp = divmod(c, 4)
        f0 = hp * F
        wqkv = np.ascontiguousarray(
            np.concatenate(
                [
                    _swizzle(Wq[:, f0 : f0 + F]),
                    _swizzle(Wk[:, f0 : f0 + F]),
                    _swizzle(Wv[:, f0 : f0 + F]),
                ],
                axis=1,
            )
        ).astype(at)
        in_maps.append(
            {
                "xT": xT[b],
                "cT": cT[b],
                "wqkv": wqkv,
                "wo": np.ascontiguousarray(Wo[f0 : f0 + F, :]).astype(
                    ml_dtypes.bfloat16
                ),
                "bo": bo.reshape(1, D) if hp == 0 else zero_bo,
            }
        )
    return in_maps


def kernel(x, context, Wq, Wk, Wv, Wo, bo):
    from concourse.bass_utils import run_bass_kernel_spmd

    in_maps = shard_inputs(x, context, Wq, Wk, Wv, Wo, bo)
    nc = _get_nc()
    res = run_bass_kernel_spmd(nc, in_maps, list(range(8)))
    out = np.zeros((2, SEQ, D), np.float32)
    for c in range(8):
        out[c // 4] += res.results[c]["out_p"]
    return out
